# revision 5
# baseline (speedup 1.0000x reference)
"""Causal dilated 1D conv (KW=4, dilation=8) as shifted matmuls on 8 TRN2 cores.

out[b,o,t] = sum_{k,c} W[o, c*4+k] * x[b, c, t + k*8 - 24]

Sharding: data-parallel over batch (16 batches -> 2 per core). Each core runs
an identical program: weights stationary in SBUF, x streamed in 512-wide time
blocks (+24 halo). Matmuls run in float32r (fp32 data, FP22 multiply) which
streams at 1 cycle/row for free-dim >= 256.

Weight-load amortization: time-blocks are processed in groups (up to 4);
within a group each (c-chunk, tap, oc) weight is loaded once and used for
one matmul per time-block (separate PSUM banks), so the PE weight-load
overhead amortizes across the group. The first groups are single-block so
the PE starts ~3.5us in, right after the first x tiles and first weight
tile land.
"""

import numpy as np

B = 16
C_IN = 512
C_OUT = 512
T = 8192
KW = 4
DIL = 8
PAD = (KW - 1) * DIL  # 24

N_CORES = 8
B_PER = B // N_CORES  # 2
P = 128
TBLK = 512
NT = T // TBLK        # 16
NCC = C_IN // P       # 4
NOC = C_OUT // P      # 4

# (batch, [time-block indices]) processing groups: small groups first so the
# PE starts on minimal DMA, then groups of 4 for weight-load amortization.
GROUPS = []
for _b in range(B_PER):
    if _b == 0:
        _sizes = [1, 1, 2, 4, 4, 4]
    else:
        _sizes = [4, 4, 4, 4]
    _tb = 0
    for _s in _sizes:
        GROUPS.append((_b, list(range(_tb, _tb + _s))))
        _tb += _s
    assert _tb == NT

_cache = {}


def _build():
    import concourse.tile as tile
    from concourse import bacc, mybir

    nc = bacc.Bacc("TRN2", target_bir_lowering=False, debug=False,
                   num_devices=N_CORES)
    x = nc.dram_tensor("x", [B_PER, C_IN, T + PAD], mybir.dt.float32r,
                       kind="ExternalInput").ap()
    # weights pre-arranged on host as [cc, tap, c=128, o=512]
    wt = nc.dram_tensor("wt", [NCC, KW, P, C_OUT], mybir.dt.float32r,
                        kind="ExternalInput").ap()
    out = nc.dram_tensor("out", [B_PER, C_OUT, T], mybir.dt.float32,
                         kind="ExternalOutput").ap()
    f32 = mybir.dt.float32
    f32r = mybir.dt.float32r

    with tile.TileContext(nc) as tc:
        with tc.tile_pool(name="wpool", bufs=1) as wpool, \
             tc.tile_pool(name="xpool", bufs=8) as xpool, \
             tc.tile_pool(name="opool", bufs=8) as opool, \
             tc.tile_pool(name="pspool", bufs=8, space="PSUM") as pspool:

            def load_xt(b, tb):
                xts = []
                for cc in range(NCC):
                    xt = xpool.tile([P, TBLK + PAD], f32r,
                                    name=f"xt{cc}", tag=f"xt{cc}")
                    nc.sync.dma_start(
                        xt[:],
                        x[b, cc * P:(cc + 1) * P,
                          tb * TBLK: tb * TBLK + TBLK + PAD])
                    xts.append(xt)
                return xts

            # First time-block's x tiles before any weights: small (1.1MB)
            # so the first weight tile lands early and the PE starts fast.
            first_xts = load_xt(0, 0)

            # Weights resident for the whole kernel: [c=128, o=512] per
            # (c-chunk, tap), issued in the order the first group consumes
            # them (cc outer, tap inner).
            wtiles = [[None] * KW for _ in range(NCC)]
            for cc in range(NCC):
                for k in range(KW):
                    wtile = wpool.tile([P, C_OUT], f32r, name=f"w_{cc}_{k}",
                                       tag=f"w_{cc}_{k}")
                    nc.sync.dma_start(wtile[:], wt[cc, k])
                    wtiles[cc][k] = wtile

            for gi, (b, tbs) in enumerate(GROUPS):
                if gi == 0:
                    gxts = [first_xts]
                else:
                    gxts = [load_xt(b, tb) for tb in tbs]
                for oc in range(NOC):
                    pss = [pspool.tile([P, TBLK], f32, name="ps", tag="ps")
                           for _ in tbs]
                    n_acc = NCC * KW
                    for ci, (cc, k) in enumerate(
                            (cc, k) for cc in range(NCC) for k in range(KW)):
                        w = wtiles[cc][k][:, oc * P:(oc + 1) * P]
                        for ti in range(len(tbs)):
                            nc.tensor.matmul(
                                pss[ti][:],
                                w,
                                gxts[ti][cc][:, k * DIL: k * DIL + TBLK],
                                start=(ci == 0),
                                stop=(ci == n_acc - 1),
                            )
                    for ti, tb in enumerate(tbs):
                        ot = opool.tile([P, TBLK], f32, name="ot", tag="ot")
                        nc.vector.tensor_copy(ot[:], pss[ti][:])
                        nc.sync.dma_start(
                            out[b, oc * P:(oc + 1) * P,
                                tb * TBLK:(tb + 1) * TBLK],
                            ot[:])

    nc.compile()
    return nc


def _get_nc():
    if "nc" not in _cache:
        _cache["nc"] = _build()
    return _cache["nc"]


def _make_in_maps(x, W):
    xpad = np.pad(np.ascontiguousarray(x, dtype=np.float32),
                  ((0, 0), (0, 0), (PAD, 0)))
    w = np.ascontiguousarray(W, dtype=np.float32).reshape(C_OUT, C_IN, KW)
    # wt[cc, k, c, o] = W[o, (cc*128+c)*KW + k]
    wt = np.transpose(w.reshape(C_OUT, NCC, P, KW), (1, 3, 2, 0)).copy()
    return [{"x": np.ascontiguousarray(xpad[i * B_PER:(i + 1) * B_PER]),
             "wt": wt} for i in range(N_CORES)]


def kernel(x, W):
    from concourse.bass_utils import run_bass_kernel_spmd

    nc = _get_nc()
    in_maps = _make_in_maps(x, W)
    res = run_bass_kernel_spmd(nc, in_maps, list(range(N_CORES)))
    return np.concatenate([r["out"] for r in res.results], axis=0)


# revision 7
# speedup vs baseline: 1.1068x; 1.1068x over previous
"""Causal dilated 1D conv (KW=4, dilation=8) as shifted matmuls on 8 TRN2 cores.

out[b,o,t] = sum_{k,c} W[o, c*4+k] * x[b, c, t + k*8 - 24]

Sharding: data-parallel over batch (16 batches -> 2 per core). Each core runs
an identical program: weights stationary in SBUF, x streamed in 512-wide time
blocks (+24 halo). Matmuls run in float32r (fp32 data, FP22 multiply) which
streams at 1 cycle/row for free-dim >= 256.

Weight-load amortization: time-blocks are processed in groups (up to 4);
within a group each (c-chunk, tap, oc) weight is loaded once and used for
one matmul per time-block (separate PSUM banks), so the PE weight-load
overhead amortizes across the group. The first groups are single-block so
the PE starts ~3.5us in, right after the first x tiles and first weight
tile land.
"""

import numpy as np

B = 16
C_IN = 512
C_OUT = 512
T = 8192
KW = 4
DIL = 8
PAD = (KW - 1) * DIL  # 24

N_CORES = 8
B_PER = B // N_CORES  # 2
P = 128
TBLK = 512
NT = T // TBLK        # 16
NCC = C_IN // P       # 4
NOC = C_OUT // P      # 4

# (batch, [time-block indices]) processing groups: small groups first so the
# PE starts on minimal DMA, then groups of 4 for weight-load amortization.
GROUPS = []
for _b in range(B_PER):
    if _b == 0:
        _sizes = [1, 1, 2, 4, 4, 4]
    else:
        _sizes = [4, 4, 4, 3, 1]
    _tb = 0
    for _s in _sizes:
        GROUPS.append((_b, list(range(_tb, _tb + _s))))
        _tb += _s
    assert _tb == NT

_cache = {}


def _build():
    import concourse.tile as tile
    from concourse import bacc, mybir

    nc = bacc.Bacc("TRN2", target_bir_lowering=False, debug=False,
                   num_devices=N_CORES)
    x = nc.dram_tensor("x", [B_PER, C_IN, T + PAD], mybir.dt.float32r,
                       kind="ExternalInput").ap()
    # weights pre-arranged on host as [cc, tap, c=128, o=512]
    wt = nc.dram_tensor("wt", [NCC, KW, P, C_OUT], mybir.dt.float32r,
                        kind="ExternalInput").ap()
    out = nc.dram_tensor("out", [B_PER, C_OUT, T], mybir.dt.float32,
                         kind="ExternalOutput").ap()
    f32 = mybir.dt.float32
    f32r = mybir.dt.float32r

    with tile.TileContext(nc) as tc:
        with tc.tile_pool(name="wpool", bufs=1) as wpool, \
             tc.tile_pool(name="xpool", bufs=8) as xpool, \
             tc.tile_pool(name="opool", bufs=8) as opool, \
             tc.tile_pool(name="pspool", bufs=8, space="PSUM") as pspool:

            def load_xt(b, tb):
                xts = []
                for cc in range(NCC):
                    xt = xpool.tile([P, TBLK + PAD], f32r,
                                    name=f"xt{cc}", tag=f"xt{cc}")
                    nc.sync.dma_start(
                        xt[:],
                        x[b, cc * P:(cc + 1) * P,
                          tb * TBLK: tb * TBLK + TBLK + PAD])
                    xts.append(xt)
                return xts

            # First time-block's x tiles before any weights: small (1.1MB)
            # so the first weight tile lands early and the PE starts fast.
            first_xts = load_xt(0, 0)

            # Weights resident for the whole kernel: [c=128, o=512] per
            # (c-chunk, tap), issued in the order the first group consumes
            # them (cc outer, tap inner).
            wtiles = [[None] * KW for _ in range(NCC)]
            for cc in range(NCC):
                for k in range(KW):
                    wtile = wpool.tile([P, C_OUT], f32r, name=f"w_{cc}_{k}",
                                       tag=f"w_{cc}_{k}")
                    nc.scalar.dma_start(wtile[:], wt[cc, k])
                    wtiles[cc][k] = wtile

            for gi, (b, tbs) in enumerate(GROUPS):
                if gi == 0:
                    gxts = [first_xts]
                else:
                    gxts = [load_xt(b, tb) for tb in tbs]
                for oc in range(NOC):
                    pss = [pspool.tile([P, TBLK], f32, name="ps", tag="ps")
                           for _ in tbs]
                    n_acc = NCC * KW
                    for ci, (cc, k) in enumerate(
                            (cc, k) for cc in range(NCC) for k in range(KW)):
                        w = wtiles[cc][k][:, oc * P:(oc + 1) * P]
                        for ti in range(len(tbs)):
                            nc.tensor.matmul(
                                pss[ti][:],
                                w,
                                gxts[ti][cc][:, k * DIL: k * DIL + TBLK],
                                start=(ci == 0),
                                stop=(ci == n_acc - 1),
                            )
                    for ti, tb in enumerate(tbs):
                        ot = opool.tile([P, TBLK], f32, name="ot", tag="ot")
                        nc.vector.tensor_copy(ot[:], pss[ti][:])
                        nc.sync.dma_start(
                            out[b, oc * P:(oc + 1) * P,
                                tb * TBLK:(tb + 1) * TBLK],
                            ot[:])

    nc.compile()
    return nc


def _get_nc():
    if "nc" not in _cache:
        _cache["nc"] = _build()
    return _cache["nc"]


def _make_in_maps(x, W):
    xpad = np.pad(np.ascontiguousarray(x, dtype=np.float32),
                  ((0, 0), (0, 0), (PAD, 0)))
    w = np.ascontiguousarray(W, dtype=np.float32).reshape(C_OUT, C_IN, KW)
    # wt[cc, k, c, o] = W[o, (cc*128+c)*KW + k]
    wt = np.transpose(w.reshape(C_OUT, NCC, P, KW), (1, 3, 2, 0)).copy()
    return [{"x": np.ascontiguousarray(xpad[i * B_PER:(i + 1) * B_PER]),
             "wt": wt} for i in range(N_CORES)]


def kernel(x, W):
    from concourse.bass_utils import run_bass_kernel_spmd

    nc = _get_nc()
    in_maps = _make_in_maps(x, W)
    res = run_bass_kernel_spmd(nc, in_maps, list(range(N_CORES)))
    return np.concatenate([r["out"] for r in res.results], axis=0)
